# revision 27
# baseline (speedup 1.0000x reference)
"""Trainium2 Bass kernel for 4-head spatial self-attention (nn_Attention).

Reference computation (shapes hardcoded):
  x [4, 256, 64, 64] --1x1conv--> qkv [4, 384, 64, 64]
  per (batch, head): sim = (q*scale)^T k over c_head=32, softmax over j,
  out = attn @ v^T, then 1x1 out-projection back to 256 channels.

Sharding: 8 cores = 4 batches x 2 spatial halves (i-halves of 2048 tokens).
Each core computes k,v for its full batch and q for its i-half, producing a
complete [256, 2048] output slice; the host just concatenates. All cores run
an identical program (only the input data differs).

Per-core kernel strategy (v3 - concurrent-PE pipeline):
  - The scalar engine's exp throughput (1 elem/lane/cycle @ 1.2 GHz) is the
    fundamental wall: 33.5M exps/core = ~220us on ACT alone.  This kernel
    splits each sim tile's exp between ACT (native Exp) and the vector
    engine (DVE), which computes a Schraudolph-style exp directly into bf16
    bits: e_bf16bits = int16(A*s + B) with A = 128*log2(e),
    B = 128*127 - 5.51; the int16 tile is bitcast to bf16 for the PV matmul.
    The +-3.3% multiplicative wiggle of the approximation washes out through
    the softmax normalization (measured end-to-end rel err stays ~4e-3).
  - sim is computed TRANSPOSED (j on partitions) via k^T q so the PV matmul
    needs no transpose; max-subtraction is skipped (logits ~N(0,1)).
  - Everything on the PE runs in bf16 (1 cyc/col, FWL weight loads): host
    pre-converts x and all weights to bf16.
  - The j-stream is processed in units of 3 chunks (one chunk = one
    (j-tile, head) = 512 sim columns) held in [128, 3*512] fp32 PSUM tiles,
    double-buffered (6 banks), with the 2 PV accumulator banks making 8.
  - Software pipelining: QK(unit u) and exp(unit u) are emitted before
    PV(unit u-1) so the in-order PE queue never blocks the exp engines; the
    per-i-block normalization/out-projection is interleaved into the first
    units of the next i-block.
  - Softmax denominator is fused into PV as a ones-column of vT (M=64
    stationary with zero padding so all PSUM rows are defined); bias is
    added via a K=1 matmul; normalization broadcast via PE ones-matmul and
    a single tensor_mul per PSUM bank.
"""

import numpy as np

HEADS = 4
C_HEAD = 32
C_IN = 256
C_HID = 128
B = 4
NJ = 4096  # full token count (64*64)
NI = 2048  # per-core i-half
IB = 512  # i-block (PSUM bank width in fp32)
NJT = NJ // 128  # 32 j-tiles
NIB = NI // IB  # 4 i-blocks
P = 128

import os

# Schraudolph exp constants: bf16 bits of e^s ~= int16(A*s + B)
EXP_A = 128 * 1.4426950408889634
EXP_B = 128.0 * 127.0 - 5.512
# fraction of each unit's columns handled by ACT (rest by DVE Schraudolph);
# balanced so both engines finish a unit at the same time.
ACT_FRAC = float(os.environ.get("K_ACT_FRAC", "1.0"))

_STATE = {}


def _build_program(reps=1):
    import concourse.bacc as bacc
    import concourse.tile as tile
    from concourse import mybir

    F32 = mybir.dt.float32
    BF16 = mybir.dt.bfloat16

    nc = bacc.Bacc(None, target_bir_lowering=False)

    xkv = nc.declare_dram_parameter("xkv", [C_IN, NJ], BF16, isOutput=False)
    xq = nc.declare_dram_parameter("xq", [C_IN, NI], BF16, isOutput=False)
    wq = nc.declare_dram_parameter("wq_t", [C_IN, C_HID], BF16, isOutput=False)
    wk = nc.declare_dram_parameter("wk_t", [C_IN, C_HID], BF16, isOutput=False)
    wv = nc.declare_dram_parameter("wv_t", [C_IN, C_HID], BF16, isOutput=False)
    woa = nc.declare_dram_parameter("woa_t", [C_HID, C_IN], BF16, isOutput=False)
    wob = nc.declare_dram_parameter("wob_t", [C_HID, C_IN], BF16, isOutput=False)
    out = nc.declare_dram_parameter("out", [C_IN, NI], F32, isOutput=True)

    with tile.TileContext(nc) as tc:
        with (
            tc.tile_pool(name="consts", bufs=1) as consts,
            tc.tile_pool(name="xpool", bufs=1) as xpool,
            tc.tile_pool(name="qkv", bufs=1) as qkv,
            tc.tile_pool(name="epool", bufs=10) as epool,
            tc.tile_pool(name="misc", bufs=1) as misc,
            tc.tile_pool(name="opool", bufs=2) as opool,
            tc.tile_pool(name="psim", bufs=2, space="PSUM") as psim,
            tc.tile_pool(name="ppv", bufs=1, space="PSUM") as ppv,
        ):
            # --- constants / weights (loaded once, outside the rep loop) ---
            wq_t = consts.tile([P, 2, C_HID], BF16)
            nc.sync.dma_start(out=wq_t, in_=wq[:].rearrange("(t p) m -> p t m", p=P))
            wk_t = consts.tile([P, 2, C_HID], BF16)
            nc.sync.dma_start(out=wk_t, in_=wk[:].rearrange("(t p) m -> p t m", p=P))
            wv_t = consts.tile([P, 2, C_HID], BF16)
            nc.sync.dma_start(out=wv_t, in_=wv[:].rearrange("(t p) m -> p t m", p=P))
            woa_t = consts.tile([P, C_IN], BF16)
            nc.sync.dma_start(out=woa_t, in_=woa[:])
            wob_t = consts.tile([P, C_IN], BF16)
            nc.sync.dma_start(out=wob_t, in_=wob[:])
            # broadcast stationary for the normalization: rows 32/96 hold
            # [ones(32) zeros(32)] so M=64 output rows are recip then 0.
            ones64 = consts.tile([P, C_HEAD], BF16)
            nc.vector.memset(ones64, 1.0)
            # persistent normalized-attention tiles; rows outside [0,97) stay
            # zero forever (never written by the per-ib mul) so the zero rows
            # of the padded out-projection weights see clean zeros.
            ao_a = consts.tile([P, IB], BF16)
            nc.vector.memset(ao_a, 0.0)
            nc.vector.memset(ao_a[96:98, :], 1.0)
            ao_b = consts.tile([P, IB], BF16)
            nc.vector.memset(ao_b, 0.0)
            # vT layout: [j-part, j-tile, head, 64]; cols 0:32 = v^T, col 32 =
            # ones (fused softmax denominator), cols 33:64 zero so the M=64 PV
            # matmul defines every PSUM row it touches.
            vT_t = qkv.tile([P, NJT, HEADS, 64], BF16)
            nc.vector.memset(vT_t[:, :, :, 33:64], 0.0)
            nc.vector.memset(vT_t[:, :, :, 32:33], 1.0)

            env = dict(
                xkv=xkv, xq=xq, out=out,
                wq_t=wq_t, wk_t=wk_t, wv_t=wv_t, woa_t=woa_t, wob_t=wob_t,
                ones64=ones64, vT_t=vT_t,
                ao_a=ao_a, ao_b=ao_b,
                xpool=xpool, qkv=qkv, epool=epool, misc=misc, opool=opool,
                psim=psim, ppv=ppv,
            )
            if reps == 1:
                _emit_body(nc, tc, mybir, env)
            else:
                with tc.For_i(0, reps, 1):
                    _emit_body(nc, tc, mybir, env)

    nc.compile()
    return nc


def _emit_body(nc, tc, mybir, env):
    F32 = mybir.dt.float32
    F32R = mybir.dt.float32r
    BF16 = mybir.dt.bfloat16
    I16 = mybir.dt.int16
    EXP = mybir.ActivationFunctionType.Exp
    MULT = mybir.AluOpType.mult
    ADD = mybir.AluOpType.add

    xkv, xq, out = env["xkv"], env["xq"], env["out"]
    wq_t, wk_t, wv_t = env["wq_t"], env["wk_t"], env["wv_t"]
    woa_t, wob_t = env["woa_t"], env["wob_t"]
    ones64, vT_t = env["ones64"], env["vT_t"]
    ao_a, ao_b = env["ao_a"], env["ao_b"]
    xpool, qkv, epool, misc, opool = (
        env["xpool"], env["qkv"], env["epool"], env["misc"], env["opool"],
    )
    psim, ppv = env["psim"], env["ppv"]

    UNIT0 = 2

    xq_t = xpool.tile([P, 2, NI], BF16)
    nc.sync.dma_start(out=xq_t, in_=xq[:].rearrange("(t p) n -> p t n", p=P))
    xkv_t = xpool.tile([P, 2, NJ], BF16)
    nc.sync.dma_start(out=xkv_t, in_=xkv[:].rearrange("(t p) n -> p t n", p=P))

    q_t = qkv.tile([P, NI], BF16)
    k_t = qkv.tile([P, NJ], BF16)

    # --- projections (q / k on ACT-copy path, vT on DVE-copy path) ---
    # q/k projections, t-major: each stationary weight tile is loaded once
    # per pass and its matmuls run at streaming rate.  Chunks are packed into
    # the standard [128, 2*512] sim-tag PSUM slots (2 chunks per slot).
    pq_t = [psim.tile([P, UNIT0, IB], F32, tag="sim", name="pq") for _ in range(2)]
    for t in range(2):
        for ci, c0 in enumerate(range(0, NI, IB)):
            nc.tensor.matmul(
                pq_t[ci // 2][:, ci % 2, :],
                wq_t[:, t, :], xq_t[:, t, c0 : c0 + IB],
                start=(t == 0), stop=(t == 1),
            )
    for ci in range(2):
        nc.scalar.copy(
            q_t[:, ci * 2 * IB : (ci + 1) * 2 * IB],
            pq_t[ci][:].rearrange("p l c -> p (l c)"),
        )
    for kg in range(2):
        pk_t = [psim.tile([P, UNIT0, IB], F32, tag="sim", name="pk") for _ in range(2)]
        for t in range(2):
            for ci in range(4):
                c0 = kg * 2048 + ci * IB
                nc.tensor.matmul(
                    pk_t[ci // 2][:, ci % 2, :],
                    wk_t[:, t, :], xkv_t[:, t, c0 : c0 + IB],
                    start=(t == 0), stop=(t == 1),
                )
        for ci in range(2):
            c0 = kg * 2048 + ci * 2 * IB
            nc.scalar.copy(
                k_t[:, c0 : c0 + 2 * IB],
                pk_t[ci][:].rearrange("p l c -> p (l c)"),
            )
    for gp in range(NJT // 4):
        pvt = psim.tile([P, 4, C_HID], F32, tag="sim")
        for j4 in range(4):
            jt = gp * 4 + j4
            for t in range(2):
                nc.tensor.matmul(
                    pvt[:, j4, :], xkv_t[:, t, jt * P : (jt + 1) * P], wv_t[:, t, :],
                    start=(t == 0), stop=(t == 1),
                )
        nc.vector.tensor_copy(
            vT_t[:, gp * 4 : (gp + 1) * 4, :, 0:C_HEAD],
            pvt[:].rearrange("p j (h c) -> p j h c", h=HEADS),
        )

    # --- attention unit pipeline ---
    # chunk = (ib, jt, h) -> 512 sim columns.  unit = 2 chunks in one
    # [128, 2*512] fp32 PSUM tile (2 banks, double-buffered = 4 banks).
    # PV accumulators: 4 banks (pair x j-half), so PSUM is exactly full.
    UNIT = 2

    state = dict(pending=None, deferred=[], pv=None, pv_ib=-1, epi=None)

    def emit_unit(u):
        chunks = u["chunks"]
        L = len(chunks)
        sim_u = psim.tile([P, UNIT, IB], F32, tag="sim")
        for idx, (ib, jt, h) in enumerate(chunks):
            nc.tensor.matmul(
                sim_u[:, idx, :],
                k_t[h * C_HEAD : (h + 1) * C_HEAD, jt * P : (jt + 1) * P],
                q_t[h * C_HEAD : (h + 1) * C_HEAD, ib * IB : (ib + 1) * IB],
                start=True, stop=True,
                tile_position=(h * C_HEAD, 0),
            )
        e_u = epool.tile([P, UNIT, IB], BF16, tag="e")
        ncols = L * IB
        xs = int(round(ncols * ACT_FRAC / 32)) * 32
        sim_f = sim_u[:].rearrange("p l c -> p (l c)")
        e_f = e_u[:].rearrange("p l c -> p (l c)")
        if xs > 0:
            nc.scalar.activation(e_f[:, 0:xs], sim_f[:, 0:xs], EXP)
        if xs < ncols:
            nc.vector.tensor_scalar(
                e_f[:, xs:ncols].bitcast(I16), sim_f[:, xs:ncols],
                EXP_A, EXP_B, MULT, ADD,
            )
        u["e"] = e_u
        return u

    def flush_pv(u):
        # PV contraction split into two K=64 j-halves on disjoint row groups
        # ({0,1} vs {2,3}) writing disjoint PSUM banks -> the two matmuls run
        # concurrently on the PE sub-arrays (same-cell collisions impossible).
        if u is None:
            return
        ib0 = u["chunks"][0][0]
        if state["pv_ib"] != ib0:
            state["pv"] = [
                [
                    ppv.tile([P, IB], F32, tag=f"pv{pair}{half}", name="pv_t")
                    for half in range(2)
                ]
                for pair in range(2)
            ]
            state["pv_ib"] = ib0
        pv = state["pv"]
        for idx, (ib, jt, h) in enumerate(u["chunks"]):
            hh = h % 2
            for half in range(2):
                nc.tensor.matmul(
                    pv[h // 2][half][64 * hh : 64 * hh + 64, :],
                    vT_t[64 * half : 64 * half + 64, jt, h, :],
                    u["e"][64 * half : 64 * half + 64, idx, :],
                    start=(jt == 0), stop=(jt == NJT - 1),
                    tile_position=(64 * half, 64 * hh),
                )

    def epilogue_steps(ib, pv_pair):
        # generator of epilogue steps for a finished i-block
        isl = slice(ib * IB, (ib + 1) * IB)
        recips, aos, merged = [], [], []
        for pair in range(2):
            tmp = misc.tile([P, IB], F32, tag=f"mrg{pair}", name=f"mrg{pair}")
            nc.scalar.copy(tmp[0:97, :], pv_pair[pair][0][0:97, :])
            s_sb = misc.tile([P, IB], F32, tag=f"s{pair}", name=f"s_sb{pair}")
            nc.vector.tensor_add(s_sb[0:97, :], tmp[0:97, :], pv_pair[pair][1][0:97, :])
            merged.append(s_sb)
        yield
        for pair in range(2):
            recip = misc.tile([P, IB], BF16, tag=f"recip{pair}", name=f"recip{pair}")
            with nc.allow_low_precision(reason="bf16 recip feeds PE broadcast"):
                nc.vector.reciprocal(recip[0:97, :], merged[pair][0:97, :])
            recips.append(recip)
        yield
        bcs_sb = []
        for pair in range(2):
            bc_ps = psim.tile([P, IB], F32, tag="sim", name=f"bc_ps{pair}")
            for hh in range(2):
                nc.tensor.matmul(
                    bc_ps[64 * hh : 64 * hh + C_HEAD, :],
                    ones64[32 + 64 * hh : 33 + 64 * hh, :],
                    recips[pair][32 + 64 * hh : 33 + 64 * hh, :],
                    start=True, stop=True,
                    tile_position=(32 + 64 * hh, 64 * hh),
                )
            bc_sb = misc.tile([P, IB], F32, tag=f"bc{pair}", name=f"bc_sb{pair}")
            nc.scalar.copy(bc_sb, bc_ps)
            bcs_sb.append(bc_sb)
        yield
        for pair, ao in enumerate((ao_a, ao_b)):
            # rows 33:64 of merged are zeros (vT zero padding), so the garbage
            # in the unwritten bc rows is multiplied by 0; rows 97:128 of ao
            # stay at their initial 0.
            nc.vector.tensor_mul(ao[0:97, :], merged[pair][0:97, :], bcs_sb[pair][0:97, :])
            aos.append(ao)
        yield
        for ot in range(2):
            pr = ppv.tile([P, IB], F32, tag=f"pv{ot}0", name=f"pr{ot}")
            osl = slice(ot * P, (ot + 1) * P)
            nc.tensor.matmul(pr, woa_t[:, osl], aos[0], start=True, stop=False)
            nc.tensor.matmul(pr, wob_t[:, osl], aos[1], start=False, stop=True)
            o_t = opool.tile([P, IB], F32, tag="o", name="o_t")
            nc.vector.tensor_scalar(o_t, pr, 1.0, None, MULT)
            nc.sync.dma_start(
                out=out[:].rearrange("(t p) n -> p t n", p=P)[:, ot, isl],
                in_=o_t,
            )
            yield

    for ib in range(NIB):
        chunks = [(ib, jt, h) for jt in range(NJT) for h in range(HEADS)]
        units = [
            {"chunks": chunks[i : i + UNIT]} for i in range(0, len(chunks), UNIT)
        ]
        for ui, u in enumerate(units):
            emit_unit(u)
            if state["epi"] is not None:
                # previous i-block's epilogue: defer PV flushes while its
                # steps interleave with our units (pv slot WAR ordering)
                state["deferred"].append(state["pending"])
                state["pending"] = None
                try:
                    next(state["epi"])
                except StopIteration:
                    state["epi"] = None
                    for d in state["deferred"]:
                        flush_pv(d)
                    state["deferred"] = []
            else:
                flush_pv(state["pending"])
            state["pending"] = u
            if ui == 0 and ib > 0:
                # all PV of ib-1 flushed (pending was its last unit just
                # flushed above); start its epilogue now
                state["epi"] = epilogue_steps(ib - 1, state["pv"])
    # tail: flush remaining work
    flush_pv(state["pending"])
    state["pending"] = None
    if state["epi"] is not None:
        for _ in state["epi"]:
            pass
        for d in state["deferred"]:
            flush_pv(d)
        state["deferred"] = []
    for _ in epilogue_steps(NIB - 1, state["pv"]):
        pass


def _get_nc(reps=1):
    key = ("nc", reps)
    if key not in _STATE:
        _STATE[key] = _build_program(reps)
    return _STATE[key]


def _to_bf16(a):
    import ml_dtypes

    return np.ascontiguousarray(a).astype(ml_dtypes.bfloat16)


def _make_in_maps(x, w_qkv, w_out, b_out):
    x = np.ascontiguousarray(x, dtype=np.float32)
    w_qkv = np.asarray(w_qkv, dtype=np.float32)
    w_out = np.asarray(w_out, dtype=np.float32)
    b_out = np.asarray(b_out, dtype=np.float32)
    scale = np.float32(C_HEAD**-0.5)
    wo_t = w_out.T  # [c_hid, c_in]
    woa = np.zeros((C_HID, C_IN), np.float32)
    wob = np.zeros((C_HID, C_IN), np.float32)
    woa[0:32] = wo_t[0:32]  # head 0
    woa[64:96] = wo_t[32:64]  # head 1
    woa[97] = b_out  # bias rides on ao_a's persistent ones-row
    wob[0:32] = wo_t[64:96]  # head 2
    wob[64:96] = wo_t[96:128]  # head 3
    shared = {
        "wq_t": _to_bf16((w_qkv[0:C_HID] * scale).T),
        "wk_t": _to_bf16(w_qkv[C_HID : 2 * C_HID].T),
        "wv_t": _to_bf16(w_qkv[2 * C_HID : 3 * C_HID].T),
        "woa_t": _to_bf16(woa),
        "wob_t": _to_bf16(wob),
    }
    in_maps = []
    for c in range(8):
        b, half = divmod(c, 2)
        xkv = _to_bf16(x[b].reshape(C_IN, NJ))
        xq = np.ascontiguousarray(xkv[:, half * NI : (half + 1) * NI])
        in_maps.append({"xkv": xkv, "xq": xq, **shared})
    return in_maps


def _assemble(results):
    out = np.empty((B, C_IN, NJ), np.float32)
    for c in range(8):
        b, half = divmod(c, 2)
        out[b][:, half * NI : (half + 1) * NI] = results[c]["out"]
    return out.reshape(B, C_IN, 64, 64)


def _run(in_maps, reps=1, **kwargs):
    from concourse.bass_utils import run_bass_kernel_spmd

    return run_bass_kernel_spmd(
        _get_nc(reps), in_maps, core_ids=list(range(8)), **kwargs
    )


def kernel(x, w_qkv, w_out, b_out):
    res = _run(_make_in_maps(x, w_qkv, w_out, b_out))
    return _assemble(res.results)


# revision 29
# speedup vs baseline: 1.2539x; 1.2539x over previous
"""Trainium2 Bass kernel for 4-head spatial self-attention (nn_Attention).

Reference computation (shapes hardcoded):
  x [4, 256, 64, 64] --1x1conv--> qkv [4, 384, 64, 64]
  per (batch, head): sim = (q*scale)^T k over c_head=32, softmax over j,
  out = attn @ v^T, then 1x1 out-projection back to 256 channels.

Sharding: 8 cores = 4 batches x 2 spatial halves (i-halves of 2048 tokens).
Each core computes k,v for its full batch and q for its i-half, producing a
complete [256, 2048] output slice; the host just concatenates. All cores run
an identical program (only the input data differs).

Per-core kernel strategy (v3 - concurrent-PE pipeline):
  - The scalar engine's exp throughput (1 elem/lane/cycle @ 1.2 GHz) is the
    fundamental wall: 33.5M exps/core = ~220us on ACT alone.  This kernel
    splits each sim tile's exp between ACT (native Exp) and the vector
    engine (DVE), which computes a Schraudolph-style exp directly into bf16
    bits: e_bf16bits = int16(A*s + B) with A = 128*log2(e),
    B = 128*127 - 5.51; the int16 tile is bitcast to bf16 for the PV matmul.
    The +-3.3% multiplicative wiggle of the approximation washes out through
    the softmax normalization (measured end-to-end rel err stays ~4e-3).
  - sim is computed TRANSPOSED (j on partitions) via k^T q so the PV matmul
    needs no transpose; max-subtraction is skipped (logits ~N(0,1)).
  - Everything on the PE runs in bf16 (1 cyc/col, FWL weight loads): host
    pre-converts x and all weights to bf16.
  - The j-stream is processed in units of 3 chunks (one chunk = one
    (j-tile, head) = 512 sim columns) held in [128, 3*512] fp32 PSUM tiles,
    double-buffered (6 banks), with the 2 PV accumulator banks making 8.
  - Software pipelining: QK(unit u) and exp(unit u) are emitted before
    PV(unit u-1) so the in-order PE queue never blocks the exp engines; the
    per-i-block normalization/out-projection is interleaved into the first
    units of the next i-block.
  - Softmax denominator is fused into PV as a ones-column of vT (M=64
    stationary with zero padding so all PSUM rows are defined); bias is
    added via a K=1 matmul; normalization broadcast via PE ones-matmul and
    a single tensor_mul per PSUM bank.
"""

import numpy as np

HEADS = 4
C_HEAD = 32
C_IN = 256
C_HID = 128
B = 4
NJ = 4096  # full token count (64*64)
NI = 2048  # per-core i-half
IB = 512  # i-block (PSUM bank width in fp32)
NJT = NJ // 128  # 32 j-tiles
NIB = NI // IB  # 4 i-blocks
P = 128

import os

# Schraudolph exp constants: bf16 bits of e^s ~= int16(A*s + B)
EXP_A = 128 * 1.4426950408889634
EXP_B = 128.0 * 127.0 - 5.512
# fraction of each unit's columns handled by ACT (rest by DVE Schraudolph);
# balanced so both engines finish a unit at the same time.
ACT_FRAC = float(os.environ.get("K_ACT_FRAC", "1.0"))

_STATE = {}


def _build_program(reps=1):
    import concourse.bacc as bacc
    import concourse.tile as tile
    from concourse import mybir

    F32 = mybir.dt.float32
    BF16 = mybir.dt.bfloat16

    nc = bacc.Bacc(None, target_bir_lowering=False)

    xkv = nc.declare_dram_parameter("xkv", [C_IN, NJ], BF16, isOutput=False)
    xq = nc.declare_dram_parameter("xq", [C_IN, NI], BF16, isOutput=False)
    wq = nc.declare_dram_parameter("wq_t", [C_IN, C_HID], BF16, isOutput=False)
    wk = nc.declare_dram_parameter("wk_t", [C_IN, C_HID], BF16, isOutput=False)
    wv = nc.declare_dram_parameter("wv_t", [C_IN, C_HID], BF16, isOutput=False)
    woa = nc.declare_dram_parameter("woa_t", [C_HID, C_IN], BF16, isOutput=False)
    wob = nc.declare_dram_parameter("wob_t", [C_HID, C_IN], BF16, isOutput=False)
    out = nc.declare_dram_parameter("out", [C_IN, NI], F32, isOutput=True)

    with tile.TileContext(nc) as tc:
        with (
            tc.tile_pool(name="consts", bufs=1) as consts,
            tc.tile_pool(name="xpool", bufs=1) as xpool,
            tc.tile_pool(name="qkv", bufs=1) as qkv,
            tc.tile_pool(name="epool", bufs=10) as epool,
            tc.tile_pool(name="misc", bufs=1) as misc,
            tc.tile_pool(name="opool", bufs=2) as opool,
            tc.tile_pool(name="psim", bufs=2, space="PSUM") as psim,
            tc.tile_pool(name="ppv", bufs=1, space="PSUM") as ppv,
        ):
            # --- constants / weights (loaded once, outside the rep loop) ---
            wq_t = consts.tile([P, 2, C_HID], BF16)
            nc.sync.dma_start(out=wq_t, in_=wq[:].rearrange("(t p) m -> p t m", p=P))
            wk_t = consts.tile([P, 2, C_HID], BF16)
            nc.sync.dma_start(out=wk_t, in_=wk[:].rearrange("(t p) m -> p t m", p=P))
            wv_t = consts.tile([P, 2, C_HID], BF16)
            nc.sync.dma_start(out=wv_t, in_=wv[:].rearrange("(t p) m -> p t m", p=P))
            woa_t = consts.tile([P, C_IN], BF16)
            nc.sync.dma_start(out=woa_t, in_=woa[:])
            wob_t = consts.tile([P, C_IN], BF16)
            nc.sync.dma_start(out=wob_t, in_=wob[:])
            # broadcast stationary for the normalization: rows 32/96 hold
            # [ones(32) zeros(32)] so M=64 output rows are recip then 0.
            ones64 = consts.tile([P, C_HEAD], BF16)
            nc.vector.memset(ones64, 1.0)
            # persistent normalized-attention tiles; rows outside [0,97) stay
            # zero forever (never written by the per-ib mul) so the zero rows
            # of the padded out-projection weights see clean zeros.
            ao_a = consts.tile([P, IB], BF16)
            nc.vector.memset(ao_a, 0.0)
            nc.vector.memset(ao_a[96:98, :], 1.0)
            ao_b = consts.tile([P, IB], BF16)
            nc.vector.memset(ao_b, 0.0)
            # vT layout: [j-part, j-tile, head, 64]; cols 0:32 = v^T, col 32 =
            # ones (fused softmax denominator), cols 33:64 zero so the M=64 PV
            # matmul defines every PSUM row it touches.
            vT_t = qkv.tile([P, NJT, HEADS, 64], BF16)
            nc.vector.memset(vT_t[:, :, :, 33:64], 0.0)
            nc.vector.memset(vT_t[:, :, :, 32:33], 1.0)

            env = dict(
                xkv=xkv, xq=xq, out=out,
                wq_t=wq_t, wk_t=wk_t, wv_t=wv_t, woa_t=woa_t, wob_t=wob_t,
                ones64=ones64, vT_t=vT_t,
                ao_a=ao_a, ao_b=ao_b,
                xpool=xpool, qkv=qkv, epool=epool, misc=misc, opool=opool,
                psim=psim, ppv=ppv,
            )
            if reps == 1:
                _emit_body(nc, tc, mybir, env)
            else:
                with tc.For_i(0, reps, 1):
                    _emit_body(nc, tc, mybir, env)

    nc.compile()
    return nc


def _emit_body(nc, tc, mybir, env):
    F32 = mybir.dt.float32
    F32R = mybir.dt.float32r
    BF16 = mybir.dt.bfloat16
    I16 = mybir.dt.int16
    EXP = mybir.ActivationFunctionType.Exp
    MULT = mybir.AluOpType.mult
    ADD = mybir.AluOpType.add

    xkv, xq, out = env["xkv"], env["xq"], env["out"]
    wq_t, wk_t, wv_t = env["wq_t"], env["wk_t"], env["wv_t"]
    woa_t, wob_t = env["woa_t"], env["wob_t"]
    ones64, vT_t = env["ones64"], env["vT_t"]
    ao_a, ao_b = env["ao_a"], env["ao_b"]
    xpool, qkv, epool, misc, opool = (
        env["xpool"], env["qkv"], env["epool"], env["misc"], env["opool"],
    )
    psim, ppv = env["psim"], env["ppv"]

    UNIT0 = 2

    # input DMAs ride the otherwise-idle Pool queue: the next rep's loads
    # issue as soon as the projections of this rep have consumed the tiles,
    # hiding the transfer entirely behind the attention stream.
    xq_t = xpool.tile([P, 2, NI], BF16)
    nc.gpsimd.dma_start(out=xq_t, in_=xq[:].rearrange("(t p) n -> p t n", p=P))
    xkv_t = xpool.tile([P, 2, NJ], BF16)
    nc.gpsimd.dma_start(out=xkv_t, in_=xkv[:].rearrange("(t p) n -> p t n", p=P))

    q_t = qkv.tile([P, NI], BF16)
    k_t = qkv.tile([P, NJ], BF16)

    # --- projections (q / k on ACT-copy path, vT on DVE-copy path) ---
    # q/k projections, t-major: each stationary weight tile is loaded once
    # per pass and its matmuls run at streaming rate.  Chunks are packed into
    # the standard [128, 2*512] sim-tag PSUM slots (2 chunks per slot).
    pq_t = [psim.tile([P, UNIT0, IB], F32, tag="sim", name="pq") for _ in range(2)]
    for t in range(2):
        for ci, c0 in enumerate(range(0, NI, IB)):
            nc.tensor.matmul(
                pq_t[ci // 2][:, ci % 2, :],
                wq_t[:, t, :], xq_t[:, t, c0 : c0 + IB],
                start=(t == 0), stop=(t == 1),
            )
    for ci in range(2):
        nc.scalar.copy(
            q_t[:, ci * 2 * IB : (ci + 1) * 2 * IB],
            pq_t[ci][:].rearrange("p l c -> p (l c)"),
        )
    for kg in range(2):
        pk_t = [psim.tile([P, UNIT0, IB], F32, tag="sim", name="pk") for _ in range(2)]
        for t in range(2):
            for ci in range(4):
                c0 = kg * 2048 + ci * IB
                nc.tensor.matmul(
                    pk_t[ci // 2][:, ci % 2, :],
                    wk_t[:, t, :], xkv_t[:, t, c0 : c0 + IB],
                    start=(t == 0), stop=(t == 1),
                )
        for ci in range(2):
            c0 = kg * 2048 + ci * 2 * IB
            nc.scalar.copy(
                k_t[:, c0 : c0 + 2 * IB],
                pk_t[ci][:].rearrange("p l c -> p (l c)"),
            )
    for gp in range(NJT // 4):
        pvt = psim.tile([P, 4, C_HID], F32, tag="sim")
        for j4 in range(4):
            jt = gp * 4 + j4
            for t in range(2):
                nc.tensor.matmul(
                    pvt[:, j4, :], xkv_t[:, t, jt * P : (jt + 1) * P], wv_t[:, t, :],
                    start=(t == 0), stop=(t == 1),
                )
        nc.vector.tensor_copy(
            vT_t[:, gp * 4 : (gp + 1) * 4, :, 0:C_HEAD],
            pvt[:].rearrange("p j (h c) -> p j h c", h=HEADS),
        )

    # --- attention unit pipeline ---
    # chunk = (ib, jt, h) -> 512 sim columns.  unit = 2 chunks in one
    # [128, 2*512] fp32 PSUM tile (2 banks, double-buffered = 4 banks).
    # PV accumulators: 4 banks (pair x j-half), so PSUM is exactly full.
    UNIT = 2

    state = dict(pending=None, deferred=[], pv=None, pv_ib=-1, epi=None)

    def emit_qk(u):
        chunks = u["chunks"]
        sim_u = psim.tile([P, UNIT, IB], F32, tag="sim")
        for idx, (ib, jt, h) in enumerate(chunks):
            nc.tensor.matmul(
                sim_u[:, idx, :],
                k_t[h * C_HEAD : (h + 1) * C_HEAD, jt * P : (jt + 1) * P],
                q_t[h * C_HEAD : (h + 1) * C_HEAD, ib * IB : (ib + 1) * IB],
                start=True, stop=True,
                tile_position=(h * C_HEAD, 0),
            )
        u["sim"] = sim_u

    def emit_exp(u):
        sim_u = u.pop("sim")
        L = len(u["chunks"])
        e_u = epool.tile([P, UNIT, IB], BF16, tag="e")
        ncols = L * IB
        xs = int(round(ncols * ACT_FRAC / 32)) * 32
        sim_f = sim_u[:].rearrange("p l c -> p (l c)")
        e_f = e_u[:].rearrange("p l c -> p (l c)")
        if xs > 0:
            nc.scalar.activation(e_f[:, 0:xs], sim_f[:, 0:xs], EXP)
        if xs < ncols:
            nc.vector.tensor_scalar(
                e_f[:, xs:ncols].bitcast(I16), sim_f[:, xs:ncols],
                EXP_A, EXP_B, MULT, ADD,
            )
        u["e"] = e_u

    def flush_pv(macro):
        # PV contraction split into two K=64 j-halves on disjoint row groups
        # ({0,1} vs {2,3}) writing disjoint PSUM banks -> the two matmuls run
        # concurrently on the PE sub-arrays (same-cell collisions impossible).
        if macro is None:
            return
        ib0 = macro[0]["chunks"][0][0]
        if state["pv_ib"] != ib0:
            state["pv"] = [
                [
                    ppv.tile([P, IB], F32, tag=f"pv{pair}{half}", name="pv_t")
                    for half in range(2)
                ]
                for pair in range(2)
            ]
            state["pv_ib"] = ib0
        pv = state["pv"]
        for u in macro:
            for idx, (ib, jt, h) in enumerate(u["chunks"]):
                hh = h % 2
                for half in range(2):
                    nc.tensor.matmul(
                        pv[h // 2][half][64 * hh : 64 * hh + 64, :],
                        vT_t[64 * half : 64 * half + 64, jt, h, :],
                        u["e"][64 * half : 64 * half + 64, idx, :],
                        start=(jt == 0), stop=(jt == NJT - 1),
                        tile_position=(64 * half, 64 * hh),
                    )

    def epilogue_steps(ib, pv_pair):
        # generator of epilogue steps for a finished i-block
        isl = slice(ib * IB, (ib + 1) * IB)
        recips, aos, merged = [], [], []
        for pair in range(2):
            tmp = misc.tile([P, IB], F32, tag=f"mrg{pair}", name=f"mrg{pair}")
            nc.scalar.copy(tmp[0:97, :], pv_pair[pair][0][0:97, :])
            s_sb = misc.tile([P, IB], F32, tag=f"s{pair}", name=f"s_sb{pair}")
            nc.vector.tensor_add(s_sb[0:97, :], tmp[0:97, :], pv_pair[pair][1][0:97, :])
            merged.append(s_sb)
        yield
        for pair in range(2):
            recip = misc.tile([P, IB], BF16, tag=f"recip{pair}", name=f"recip{pair}")
            with nc.allow_low_precision(reason="bf16 recip feeds PE broadcast"):
                nc.vector.reciprocal(recip[0:97, :], merged[pair][0:97, :])
            recips.append(recip)
        yield
        bcs_sb = []
        for pair in range(2):
            bc_ps = psim.tile([P, IB], F32, tag="sim", name=f"bc_ps{pair}")
            for hh in range(2):
                nc.tensor.matmul(
                    bc_ps[64 * hh : 64 * hh + C_HEAD, :],
                    ones64[32 + 64 * hh : 33 + 64 * hh, :],
                    recips[pair][32 + 64 * hh : 33 + 64 * hh, :],
                    start=True, stop=True,
                    tile_position=(32 + 64 * hh, 64 * hh),
                )
            bc_sb = misc.tile([P, IB], F32, tag=f"bc{pair}", name=f"bc_sb{pair}")
            nc.scalar.copy(bc_sb, bc_ps)
            bcs_sb.append(bc_sb)
        yield
        for pair, ao in enumerate((ao_a, ao_b)):
            # rows 33:64 of merged are zeros (vT zero padding), so the garbage
            # in the unwritten bc rows is multiplied by 0; rows 97:128 of ao
            # stay at their initial 0.
            nc.vector.tensor_mul(ao[0:97, :], merged[pair][0:97, :], bcs_sb[pair][0:97, :])
            aos.append(ao)
        yield
        for ot in range(2):
            pr = ppv.tile([P, IB], F32, tag=f"pv{ot}0", name=f"pr{ot}")
            osl = slice(ot * P, (ot + 1) * P)
            nc.tensor.matmul(pr, woa_t[:, osl], aos[0], start=True, stop=False)
            nc.tensor.matmul(pr, wob_t[:, osl], aos[1], start=False, stop=True)
            o_t = opool.tile([P, IB], F32, tag="o", name="o_t")
            nc.vector.tensor_scalar(o_t, pr, 1.0, None, MULT)
            nc.sync.dma_start(
                out=out[:].rearrange("(t p) n -> p t n", p=P)[:, ot, isl],
                in_=o_t,
            )
            yield

    for ib in range(NIB):
        chunks = [(ib, jt, h) for jt in range(NJT) for h in range(HEADS)]
        units = [
            {"chunks": chunks[i : i + UNIT]} for i in range(0, len(chunks), UNIT)
        ]
        # macro-units: emit QK for two units back-to-back (4 heads on 4
        # distinct PE row groups -> 4-way concurrent), then their exps, then
        # the previous macro's PV flushes.
        for mi in range(0, len(units), 2):
            macro = units[mi : mi + 2]
            for u in macro:
                emit_qk(u)
            for u in macro:
                emit_exp(u)
            if state["epi"] is not None:
                # previous i-block's epilogue: defer PV flushes while its
                # steps interleave with our units (pv slot WAR ordering)
                state["deferred"].append(state["pending"])
                state["pending"] = None
                try:
                    next(state["epi"])
                except StopIteration:
                    state["epi"] = None
                    for d in state["deferred"]:
                        flush_pv(d)
                    state["deferred"] = []
            else:
                flush_pv(state["pending"])
            state["pending"] = macro
            if mi == 0 and ib > 0:
                # all PV of ib-1 flushed (pending was its last macro just
                # flushed above); start its epilogue now
                state["epi"] = epilogue_steps(ib - 1, state["pv"])
    # tail: flush remaining work
    flush_pv(state["pending"])
    state["pending"] = None
    if state["epi"] is not None:
        for _ in state["epi"]:
            pass
        for d in state["deferred"]:
            flush_pv(d)
        state["deferred"] = []
    for _ in epilogue_steps(NIB - 1, state["pv"]):
        pass


def _get_nc(reps=1):
    key = ("nc", reps)
    if key not in _STATE:
        _STATE[key] = _build_program(reps)
    return _STATE[key]


def _to_bf16(a):
    import ml_dtypes

    return np.ascontiguousarray(a).astype(ml_dtypes.bfloat16)


def _make_in_maps(x, w_qkv, w_out, b_out):
    x = np.ascontiguousarray(x, dtype=np.float32)
    w_qkv = np.asarray(w_qkv, dtype=np.float32)
    w_out = np.asarray(w_out, dtype=np.float32)
    b_out = np.asarray(b_out, dtype=np.float32)
    scale = np.float32(C_HEAD**-0.5)
    wo_t = w_out.T  # [c_hid, c_in]
    woa = np.zeros((C_HID, C_IN), np.float32)
    wob = np.zeros((C_HID, C_IN), np.float32)
    woa[0:32] = wo_t[0:32]  # head 0
    woa[64:96] = wo_t[32:64]  # head 1
    woa[97] = b_out  # bias rides on ao_a's persistent ones-row
    wob[0:32] = wo_t[64:96]  # head 2
    wob[64:96] = wo_t[96:128]  # head 3
    shared = {
        "wq_t": _to_bf16((w_qkv[0:C_HID] * scale).T),
        "wk_t": _to_bf16(w_qkv[C_HID : 2 * C_HID].T),
        "wv_t": _to_bf16(w_qkv[2 * C_HID : 3 * C_HID].T),
        "woa_t": _to_bf16(woa),
        "wob_t": _to_bf16(wob),
    }
    in_maps = []
    for c in range(8):
        b, half = divmod(c, 2)
        xkv = _to_bf16(x[b].reshape(C_IN, NJ))
        xq = np.ascontiguousarray(xkv[:, half * NI : (half + 1) * NI])
        in_maps.append({"xkv": xkv, "xq": xq, **shared})
    return in_maps


def _assemble(results):
    out = np.empty((B, C_IN, NJ), np.float32)
    for c in range(8):
        b, half = divmod(c, 2)
        out[b][:, half * NI : (half + 1) * NI] = results[c]["out"]
    return out.reshape(B, C_IN, 64, 64)


def _run(in_maps, reps=1, **kwargs):
    from concourse.bass_utils import run_bass_kernel_spmd

    return run_bass_kernel_spmd(
        _get_nc(reps), in_maps, core_ids=list(range(8)), **kwargs
    )


def kernel(x, w_qkv, w_out, b_out):
    res = _run(_make_in_maps(x, w_qkv, w_out, b_out))
    return _assemble(res.results)


# revision 30
# speedup vs baseline: 2.3493x; 1.8736x over previous
"""Trainium2 Bass kernel for 4-head spatial self-attention (nn_Attention).

Reference computation (shapes hardcoded):
  x [4, 256, 64, 64] --1x1conv--> qkv [4, 384, 64, 64]
  per (batch, head): sim = (q*scale)^T k over c_head=32, softmax over j,
  out = attn @ v^T, then 1x1 out-projection back to 256 channels.

Sharding: 8 cores = 4 batches x 2 spatial halves (i-halves of 2048 tokens).
Each core computes k,v for its full batch and q for its i-half, producing a
complete [256, 2048] output slice; the host just concatenates. All cores run
an identical program (only the input data differs).

Per-core kernel strategy (v3 - concurrent-PE pipeline):
  - The scalar engine's exp throughput (1 elem/lane/cycle @ 1.2 GHz) is the
    fundamental wall: 33.5M exps/core = ~220us on ACT alone.  This kernel
    splits each sim tile's exp between ACT (native Exp) and the vector
    engine (DVE), which computes a Schraudolph-style exp directly into bf16
    bits: e_bf16bits = int16(A*s + B) with A = 128*log2(e),
    B = 128*127 - 5.51; the int16 tile is bitcast to bf16 for the PV matmul.
    The +-3.3% multiplicative wiggle of the approximation washes out through
    the softmax normalization (measured end-to-end rel err stays ~4e-3).
  - sim is computed TRANSPOSED (j on partitions) via k^T q so the PV matmul
    needs no transpose; max-subtraction is skipped (logits ~N(0,1)).
  - Everything on the PE runs in bf16 (1 cyc/col, FWL weight loads): host
    pre-converts x and all weights to bf16.
  - The j-stream is processed in units of 3 chunks (one chunk = one
    (j-tile, head) = 512 sim columns) held in [128, 3*512] fp32 PSUM tiles,
    double-buffered (6 banks), with the 2 PV accumulator banks making 8.
  - Software pipelining: QK(unit u) and exp(unit u) are emitted before
    PV(unit u-1) so the in-order PE queue never blocks the exp engines; the
    per-i-block normalization/out-projection is interleaved into the first
    units of the next i-block.
  - Softmax denominator is fused into PV as a ones-column of vT (M=64
    stationary with zero padding so all PSUM rows are defined); bias is
    added via a K=1 matmul; normalization broadcast via PE ones-matmul and
    a single tensor_mul per PSUM bank.
"""

import numpy as np

HEADS = 4
C_HEAD = 32
C_IN = 256
C_HID = 128
B = 4
NJ = 4096  # full token count (64*64)
NI = 2048  # per-core i-half
IB = 512  # i-block (PSUM bank width in fp32)
NJT = NJ // 128  # 32 j-tiles
NIB = NI // IB  # 4 i-blocks
P = 128

import os

# Schraudolph exp constants: bf16 bits of e^s ~= int16(A*s + B)
EXP_A = 128 * 1.4426950408889634
EXP_B = 128.0 * 127.0 - 5.512
# fraction of each unit's columns handled by ACT (rest by DVE Schraudolph);
# balanced so both engines finish a unit at the same time.
ACT_FRAC = float(os.environ.get("K_ACT_FRAC", "1.0"))

_STATE = {}


def _build_program(reps=1):
    import concourse.bacc as bacc
    import concourse.tile as tile
    from concourse import mybir

    F32 = mybir.dt.float32
    BF16 = mybir.dt.bfloat16

    nc = bacc.Bacc(None, target_bir_lowering=False)

    xkv = nc.declare_dram_parameter("xkv", [C_IN, NJ], BF16, isOutput=False)
    xq = nc.declare_dram_parameter("xq", [C_IN, NI], BF16, isOutput=False)
    wq = nc.declare_dram_parameter("wq_t", [C_IN, C_HID], BF16, isOutput=False)
    wk = nc.declare_dram_parameter("wk_t", [C_IN, C_HID], BF16, isOutput=False)
    wv = nc.declare_dram_parameter("wv_t", [C_IN, C_HID], BF16, isOutput=False)
    woa = nc.declare_dram_parameter("woa_t", [C_HID, C_IN], BF16, isOutput=False)
    wob = nc.declare_dram_parameter("wob_t", [C_HID, C_IN], BF16, isOutput=False)
    out = nc.declare_dram_parameter("out", [C_IN, NI], F32, isOutput=True)

    with tile.TileContext(nc) as tc:
        with (
            tc.tile_pool(name="consts", bufs=1) as consts,
            tc.tile_pool(name="xpool", bufs=1) as xpool,
            tc.tile_pool(name="qkv", bufs=1) as qkv,
            tc.tile_pool(name="epool", bufs=16) as epool,
            tc.tile_pool(name="misc", bufs=1) as misc,
            tc.tile_pool(name="opool", bufs=2) as opool,
            tc.tile_pool(name="psim", bufs=2, space="PSUM") as psim,
            tc.tile_pool(name="ppv", bufs=1, space="PSUM") as ppv,
        ):
            # --- constants / weights (loaded once, outside the rep loop) ---
            wq_t = consts.tile([P, 2, C_HID], BF16)
            nc.sync.dma_start(out=wq_t, in_=wq[:].rearrange("(t p) m -> p t m", p=P))
            wk_t = consts.tile([P, 2, C_HID], BF16)
            nc.sync.dma_start(out=wk_t, in_=wk[:].rearrange("(t p) m -> p t m", p=P))
            wv_t = consts.tile([P, 2, C_HID], BF16)
            nc.sync.dma_start(out=wv_t, in_=wv[:].rearrange("(t p) m -> p t m", p=P))
            woa_t = consts.tile([P, C_IN], BF16)
            nc.sync.dma_start(out=woa_t, in_=woa[:])
            wob_t = consts.tile([P, C_IN], BF16)
            nc.sync.dma_start(out=wob_t, in_=wob[:])
            # broadcast stationary for the normalization: rows 32/96 hold
            # [ones(32) zeros(32)] so M=64 output rows are recip then 0.
            ones64 = consts.tile([P, C_HEAD], BF16)
            nc.vector.memset(ones64, 1.0)
            # persistent normalized-attention tiles; rows outside [0,97) stay
            # zero forever (never written by the per-ib mul) so the zero rows
            # of the padded out-projection weights see clean zeros.
            ao_a = consts.tile([P, IB], BF16)
            nc.vector.memset(ao_a, 0.0)
            nc.vector.memset(ao_a[96:98, :], 1.0)
            ao_b = consts.tile([P, IB], BF16)
            nc.vector.memset(ao_b, 0.0)
            # vT layout: [j-part, j-tile, head, 64]; cols 0:32 = v^T, col 32 =
            # ones (fused softmax denominator), cols 33:64 zero so the M=64 PV
            # matmul defines every PSUM row it touches.
            vT_t = qkv.tile([P, NJT, HEADS, 64], BF16)
            nc.vector.memset(vT_t[:, :, :, 33:64], 0.0)
            nc.vector.memset(vT_t[:, :, :, 32:33], 1.0)

            env = dict(
                xkv=xkv, xq=xq, out=out,
                wq_t=wq_t, wk_t=wk_t, wv_t=wv_t, woa_t=woa_t, wob_t=wob_t,
                ones64=ones64, vT_t=vT_t,
                ao_a=ao_a, ao_b=ao_b,
                xpool=xpool, qkv=qkv, epool=epool, misc=misc, opool=opool,
                psim=psim, ppv=ppv,
            )
            if reps == 1:
                _emit_body(nc, tc, mybir, env)
            else:
                with tc.For_i(0, reps, 1):
                    _emit_body(nc, tc, mybir, env)

    nc.compile()
    return nc


def _emit_body(nc, tc, mybir, env):
    F32 = mybir.dt.float32
    F32R = mybir.dt.float32r
    BF16 = mybir.dt.bfloat16
    I16 = mybir.dt.int16
    EXP = mybir.ActivationFunctionType.Exp
    MULT = mybir.AluOpType.mult
    ADD = mybir.AluOpType.add

    xkv, xq, out = env["xkv"], env["xq"], env["out"]
    wq_t, wk_t, wv_t = env["wq_t"], env["wk_t"], env["wv_t"]
    woa_t, wob_t = env["woa_t"], env["wob_t"]
    ones64, vT_t = env["ones64"], env["vT_t"]
    ao_a, ao_b = env["ao_a"], env["ao_b"]
    xpool, qkv, epool, misc, opool = (
        env["xpool"], env["qkv"], env["epool"], env["misc"], env["opool"],
    )
    psim, ppv = env["psim"], env["ppv"]

    UNIT0 = 2

    # input DMAs ride the otherwise-idle Pool queue: the next rep's loads
    # issue as soon as the projections of this rep have consumed the tiles,
    # hiding the transfer entirely behind the attention stream.
    xq_t = xpool.tile([P, 2, NI], BF16)
    nc.gpsimd.dma_start(out=xq_t, in_=xq[:].rearrange("(t p) n -> p t n", p=P))
    xkv_t = xpool.tile([P, 2, NJ], BF16)
    nc.gpsimd.dma_start(out=xkv_t, in_=xkv[:].rearrange("(t p) n -> p t n", p=P))

    q_t = qkv.tile([P, NI], BF16)
    k_t = qkv.tile([P, NJ], BF16)

    # --- projections (q / k on ACT-copy path, vT on DVE-copy path) ---
    # q/k projections, t-major: each stationary weight tile is loaded once
    # per pass and its matmuls run at streaming rate.  Chunks are packed into
    # the standard [128, 2*512] sim-tag PSUM slots (2 chunks per slot).
    pq_t = [psim.tile([P, UNIT0, IB], F32, tag="sim", name="pq") for _ in range(2)]
    for t in range(2):
        for ci, c0 in enumerate(range(0, NI, IB)):
            nc.tensor.matmul(
                pq_t[ci // 2][:, ci % 2, :],
                wq_t[:, t, :], xq_t[:, t, c0 : c0 + IB],
                start=(t == 0), stop=(t == 1),
            )
    for ci in range(2):
        nc.scalar.copy(
            q_t[:, ci * 2 * IB : (ci + 1) * 2 * IB],
            pq_t[ci][:].rearrange("p l c -> p (l c)"),
        )
    for kg in range(2):
        pk_t = [psim.tile([P, UNIT0, IB], F32, tag="sim", name="pk") for _ in range(2)]
        for t in range(2):
            for ci in range(4):
                c0 = kg * 2048 + ci * IB
                nc.tensor.matmul(
                    pk_t[ci // 2][:, ci % 2, :],
                    wk_t[:, t, :], xkv_t[:, t, c0 : c0 + IB],
                    start=(t == 0), stop=(t == 1),
                )
        for ci in range(2):
            c0 = kg * 2048 + ci * 2 * IB
            nc.scalar.copy(
                k_t[:, c0 : c0 + 2 * IB],
                pk_t[ci][:].rearrange("p l c -> p (l c)"),
            )
    for gp in range(NJT // 4):
        pvt = psim.tile([P, 4, C_HID], F32, tag="sim")
        for j4 in range(4):
            jt = gp * 4 + j4
            for t in range(2):
                nc.tensor.matmul(
                    pvt[:, j4, :], xkv_t[:, t, jt * P : (jt + 1) * P], wv_t[:, t, :],
                    start=(t == 0), stop=(t == 1),
                )
        nc.vector.tensor_copy(
            vT_t[:, gp * 4 : (gp + 1) * 4, :, 0:C_HEAD],
            pvt[:].rearrange("p j (h c) -> p j h c", h=HEADS),
        )

    # --- attention unit pipeline ---
    # chunk = (ib, jt, h) -> 512 sim columns.  unit = 2 chunks in one
    # [128, 2*512] fp32 PSUM tile (2 banks, double-buffered = 4 banks).
    # PV accumulators: 4 banks (pair x j-half), so PSUM is exactly full.
    UNIT = 2

    state = dict(pending=None, deferred=[], pv=None, pv_ib=-1, epi=None)

    def emit_qk(u):
        chunks = u["chunks"]
        sim_u = psim.tile([P, UNIT, IB], F32, tag="sim")
        for idx, (ib, jt, h) in enumerate(chunks):
            nc.tensor.matmul(
                sim_u[:, idx, :],
                k_t[h * C_HEAD : (h + 1) * C_HEAD, jt * P : (jt + 1) * P],
                q_t[h * C_HEAD : (h + 1) * C_HEAD, ib * IB : (ib + 1) * IB],
                start=True, stop=True,
                tile_position=(h * C_HEAD, 0),
            )
        u["sim"] = sim_u

    def emit_exp(u):
        sim_u = u.pop("sim")
        L = len(u["chunks"])
        e_u = epool.tile([P, UNIT, IB], BF16, tag="e")
        ncols = L * IB
        xs = int(round(ncols * ACT_FRAC / 32)) * 32
        sim_f = sim_u[:].rearrange("p l c -> p (l c)")
        e_f = e_u[:].rearrange("p l c -> p (l c)")
        if xs > 0:
            nc.scalar.activation(e_f[:, 0:xs], sim_f[:, 0:xs], EXP)
        if xs < ncols:
            nc.vector.tensor_scalar(
                e_f[:, xs:ncols].bitcast(I16), sim_f[:, xs:ncols],
                EXP_A, EXP_B, MULT, ADD,
            )
        u["e"] = e_u

    def flush_pv(macro):
        # PV contraction split into two K=64 j-halves on disjoint row groups
        # ({0,1} vs {2,3}) writing disjoint PSUM banks -> the two matmuls run
        # concurrently on the PE sub-arrays (same-cell collisions impossible).
        if macro is None:
            return
        ib0 = macro[0]["chunks"][0][0]
        if state["pv_ib"] != ib0:
            state["pv"] = [
                [
                    ppv.tile([P, IB], F32, tag=f"pv{pair}{half}", name="pv_t")
                    for half in range(2)
                ]
                for pair in range(2)
            ]
            state["pv_ib"] = ib0
        pv = state["pv"]
        for u in macro:
            for idx, (ib, jt, h) in enumerate(u["chunks"]):
                hh = h % 2
                for half in range(2):
                    nc.tensor.matmul(
                        pv[h // 2][half][64 * hh : 64 * hh + 64, :],
                        vT_t[64 * half : 64 * half + 64, jt, h, :],
                        u["e"][64 * half : 64 * half + 64, idx, :],
                        start=(jt == 0), stop=(jt == NJT - 1),
                        tile_position=(64 * half, 64 * hh),
                    )

    def epilogue_steps(ib, pv_pair):
        # generator of epilogue steps for a finished i-block
        isl = slice(ib * IB, (ib + 1) * IB)
        recips, aos, merged = [], [], []
        for pair in range(2):
            tmp = misc.tile([P, IB], F32, tag=f"mrg{pair}", name=f"mrg{pair}")
            nc.scalar.copy(tmp[0:97, :], pv_pair[pair][0][0:97, :])
            s_sb = misc.tile([P, IB], F32, tag=f"s{pair}", name=f"s_sb{pair}")
            nc.vector.tensor_add(s_sb[0:97, :], tmp[0:97, :], pv_pair[pair][1][0:97, :])
            merged.append(s_sb)
        yield
        for pair in range(2):
            recip = misc.tile([P, IB], BF16, tag=f"recip{pair}", name=f"recip{pair}")
            with nc.allow_low_precision(reason="bf16 recip feeds PE broadcast"):
                nc.vector.reciprocal(recip[0:97, :], merged[pair][0:97, :])
            recips.append(recip)
        yield
        bcs_sb = []
        for pair in range(2):
            bc_ps = psim.tile([P, IB], F32, tag="sim", name=f"bc_ps{pair}")
            for hh in range(2):
                nc.tensor.matmul(
                    bc_ps[64 * hh : 64 * hh + C_HEAD, :],
                    ones64[32 + 64 * hh : 33 + 64 * hh, :],
                    recips[pair][32 + 64 * hh : 33 + 64 * hh, :],
                    start=True, stop=True,
                    tile_position=(32 + 64 * hh, 64 * hh),
                )
            bc_sb = misc.tile([P, IB], F32, tag=f"bc{pair}", name=f"bc_sb{pair}")
            nc.scalar.copy(bc_sb, bc_ps)
            bcs_sb.append(bc_sb)
        yield
        for pair, ao in enumerate((ao_a, ao_b)):
            # rows 33:64 of merged are zeros (vT zero padding), so the garbage
            # in the unwritten bc rows is multiplied by 0; rows 97:128 of ao
            # stay at their initial 0.
            nc.vector.tensor_mul(ao[0:97, :], merged[pair][0:97, :], bcs_sb[pair][0:97, :])
            aos.append(ao)
        yield
        for ot in range(2):
            pr = ppv.tile([P, IB], F32, tag=f"pv{ot}0", name=f"pr{ot}")
            osl = slice(ot * P, (ot + 1) * P)
            nc.tensor.matmul(pr, woa_t[:, osl], aos[0], start=True, stop=False)
            nc.tensor.matmul(pr, wob_t[:, osl], aos[1], start=False, stop=True)
            o_t = opool.tile([P, IB], F32, tag="o", name="o_t")
            nc.vector.tensor_scalar(o_t, pr, 1.0, None, MULT)
            nc.sync.dma_start(
                out=out[:].rearrange("(t p) n -> p t n", p=P)[:, ot, isl],
                in_=o_t,
            )
            yield

    for ib in range(NIB):
        chunks = [(ib, jt, h) for jt in range(NJT) for h in range(HEADS)]
        units = [
            {"chunks": chunks[i : i + UNIT]} for i in range(0, len(chunks), UNIT)
        ]
        # macro-units: emit QK for two units back-to-back (4 heads on 4
        # distinct PE row groups -> 4-way concurrent), then their exps, then
        # the previous macro's PV flushes.
        for mi in range(0, len(units), 2):
            macro = units[mi : mi + 2]
            for u in macro:
                emit_qk(u)
            for u in macro:
                emit_exp(u)
            if state["epi"] is not None:
                # previous i-block's epilogue: defer PV flushes while its
                # steps interleave with our units (pv slot WAR ordering)
                state["deferred"].append(state["pending"])
                state["pending"] = None
                try:
                    next(state["epi"])
                except StopIteration:
                    state["epi"] = None
                    for d in state["deferred"]:
                        flush_pv(d)
                    state["deferred"] = []
            else:
                flush_pv(state["pending"])
            state["pending"] = macro
            if mi == 0 and ib > 0:
                # all PV of ib-1 flushed (pending was its last macro just
                # flushed above); start its epilogue now
                state["epi"] = epilogue_steps(ib - 1, state["pv"])
    # tail: flush remaining work
    flush_pv(state["pending"])
    state["pending"] = None
    if state["epi"] is not None:
        for _ in state["epi"]:
            pass
        for d in state["deferred"]:
            flush_pv(d)
        state["deferred"] = []
    for _ in epilogue_steps(NIB - 1, state["pv"]):
        pass


def _get_nc(reps=1):
    key = ("nc", reps)
    if key not in _STATE:
        _STATE[key] = _build_program(reps)
    return _STATE[key]


def _to_bf16(a):
    import ml_dtypes

    return np.ascontiguousarray(a).astype(ml_dtypes.bfloat16)


def _make_in_maps(x, w_qkv, w_out, b_out):
    x = np.ascontiguousarray(x, dtype=np.float32)
    w_qkv = np.asarray(w_qkv, dtype=np.float32)
    w_out = np.asarray(w_out, dtype=np.float32)
    b_out = np.asarray(b_out, dtype=np.float32)
    scale = np.float32(C_HEAD**-0.5)
    wo_t = w_out.T  # [c_hid, c_in]
    woa = np.zeros((C_HID, C_IN), np.float32)
    wob = np.zeros((C_HID, C_IN), np.float32)
    woa[0:32] = wo_t[0:32]  # head 0
    woa[64:96] = wo_t[32:64]  # head 1
    woa[97] = b_out  # bias rides on ao_a's persistent ones-row
    wob[0:32] = wo_t[64:96]  # head 2
    wob[64:96] = wo_t[96:128]  # head 3
    shared = {
        "wq_t": _to_bf16((w_qkv[0:C_HID] * scale).T),
        "wk_t": _to_bf16(w_qkv[C_HID : 2 * C_HID].T),
        "wv_t": _to_bf16(w_qkv[2 * C_HID : 3 * C_HID].T),
        "woa_t": _to_bf16(woa),
        "wob_t": _to_bf16(wob),
    }
    in_maps = []
    for c in range(8):
        b, half = divmod(c, 2)
        xkv = _to_bf16(x[b].reshape(C_IN, NJ))
        xq = np.ascontiguousarray(xkv[:, half * NI : (half + 1) * NI])
        in_maps.append({"xkv": xkv, "xq": xq, **shared})
    return in_maps


def _assemble(results):
    out = np.empty((B, C_IN, NJ), np.float32)
    for c in range(8):
        b, half = divmod(c, 2)
        out[b][:, half * NI : (half + 1) * NI] = results[c]["out"]
    return out.reshape(B, C_IN, 64, 64)


def _run(in_maps, reps=1, **kwargs):
    from concourse.bass_utils import run_bass_kernel_spmd

    return run_bass_kernel_spmd(
        _get_nc(reps), in_maps, core_ids=list(range(8)), **kwargs
    )


def kernel(x, w_qkv, w_out, b_out):
    res = _run(_make_in_maps(x, w_qkv, w_out, b_out))
    return _assemble(res.results)


# revision 31
# speedup vs baseline: 5.6784x; 2.4171x over previous
"""Trainium2 Bass kernel for 4-head spatial self-attention (nn_Attention).

Reference computation (shapes hardcoded):
  x [4, 256, 64, 64] --1x1conv--> qkv [4, 384, 64, 64]
  per (batch, head): sim = (q*scale)^T k over c_head=32, softmax over j,
  out = attn @ v^T, then 1x1 out-projection back to 256 channels.

Sharding: 8 cores = 4 batches x 2 spatial halves (i-halves of 2048 tokens).
Each core computes k,v for its full batch and q for its i-half, producing a
complete [256, 2048] output slice; the host just concatenates. All cores run
an identical program (only the input data differs).

Per-core kernel strategy (v3 - concurrent-PE pipeline):
  - All matmuls run in bf16 (host pre-converts x and weights); sim is
    computed TRANSPOSED (j on partitions) via k^T q so PV needs no
    transpose; softmax max-subtraction is skipped (logits ~N(0,1)).
  - The PE is the bottleneck engine; every hot matmul is shaped so that
    consecutive instructions land on disjoint 32-row groups of the 128x128
    array and execute CONCURRENTLY (tile_position sub-array parallelism):
      * QK: K=32 row-tiled per head; macro-units of 4 chunks put all four
        heads in flight at once.
      * PV: the K=128 j-contraction is split into two K=64 halves on row
        groups {0,1}/{2,3} accumulating into disjoint PSUM banks (no
        same-cell write races); the halves are summed during the epilogue.
      * full-K matmuls with rotating stationary weights cost ~3x streaming
        rate (drain + LDWEIGHTS serialization), so q/k projections are
        emitted t-major to keep each stationary loaded across 4 chunks.
  - exp runs on the scalar engine (2 elem/lane/cyc with bf16 output,
    measured); an optional DVE Schraudolph lane (int16(A*s+B) bitcast to
    bf16) can take a column share via K_ACT_FRAC < 1 if ACT ever binds.
  - PSUM: 2 double-buffered [128, 2*512] sim tiles (4 banks) + 4 PV
    accumulator banks = all 8 banks.
  - Softmax denominator is fused into PV as a ones-column of vT (M=64
    stationary, zero padded); the out-projection bias rides a persistent
    ones-row of ao_a through zero-padded split weights; normalization uses
    a PE ones-matmul broadcast and one tensor_mul per pair.
  - Input DMAs use the idle Pool queue so the next rep's loads prefetch
    behind the current rep's compute; per-i-block epilogues interleave into
    the first macro-units of the next i-block.
"""

import numpy as np

HEADS = 4
C_HEAD = 32
C_IN = 256
C_HID = 128
B = 4
NJ = 4096  # full token count (64*64)
NI = 2048  # per-core i-half
IB = 512  # i-block (PSUM bank width in fp32)
NJT = NJ // 128  # 32 j-tiles
NIB = NI // IB  # 4 i-blocks
P = 128

import os

# Schraudolph exp constants: bf16 bits of e^s ~= int16(A*s + B)
EXP_A = 128 * 1.4426950408889634
EXP_B = 128.0 * 127.0 - 5.512
# fraction of each unit's columns handled by ACT (rest by DVE Schraudolph);
# balanced so both engines finish a unit at the same time.
ACT_FRAC = float(os.environ.get("K_ACT_FRAC", "1.0"))

_STATE = {}


def _build_program(reps=1):
    import concourse.bacc as bacc
    import concourse.tile as tile
    from concourse import mybir

    F32 = mybir.dt.float32
    BF16 = mybir.dt.bfloat16

    nc = bacc.Bacc(None, target_bir_lowering=False)

    xkv = nc.declare_dram_parameter("xkv", [C_IN, NJ], BF16, isOutput=False)
    xq = nc.declare_dram_parameter("xq", [C_IN, NI], BF16, isOutput=False)
    wq = nc.declare_dram_parameter("wq_t", [C_IN, C_HID], BF16, isOutput=False)
    wk = nc.declare_dram_parameter("wk_t", [C_IN, C_HID], BF16, isOutput=False)
    wv = nc.declare_dram_parameter("wv_t", [C_IN, C_HID], BF16, isOutput=False)
    woa = nc.declare_dram_parameter("woa_t", [C_HID, C_IN], BF16, isOutput=False)
    wob = nc.declare_dram_parameter("wob_t", [C_HID, C_IN], BF16, isOutput=False)
    out = nc.declare_dram_parameter("out", [C_IN, NI], F32, isOutput=True)

    with tile.TileContext(nc) as tc:
        with (
            tc.tile_pool(name="consts", bufs=1) as consts,
            tc.tile_pool(name="xpool", bufs=1) as xpool,
            tc.tile_pool(name="qkv", bufs=1) as qkv,
            tc.tile_pool(name="epool", bufs=16) as epool,
            tc.tile_pool(name="misc", bufs=1) as misc,
            tc.tile_pool(name="opool", bufs=2) as opool,
            tc.tile_pool(name="psim", bufs=2, space="PSUM") as psim,
            tc.tile_pool(name="ppv", bufs=1, space="PSUM") as ppv,
        ):
            # --- constants / weights (loaded once, outside the rep loop) ---
            wq_t = consts.tile([P, 2, C_HID], BF16)
            nc.sync.dma_start(out=wq_t, in_=wq[:].rearrange("(t p) m -> p t m", p=P))
            wk_t = consts.tile([P, 2, C_HID], BF16)
            nc.sync.dma_start(out=wk_t, in_=wk[:].rearrange("(t p) m -> p t m", p=P))
            wv_t = consts.tile([P, 2, C_HID], BF16)
            nc.sync.dma_start(out=wv_t, in_=wv[:].rearrange("(t p) m -> p t m", p=P))
            woa_t = consts.tile([P, C_IN], BF16)
            nc.sync.dma_start(out=woa_t, in_=woa[:])
            wob_t = consts.tile([P, C_IN], BF16)
            nc.sync.dma_start(out=wob_t, in_=wob[:])
            # broadcast stationary for the normalization: rows 32/96 hold
            # [ones(32) zeros(32)] so M=64 output rows are recip then 0.
            ones64 = consts.tile([P, C_HEAD], BF16)
            nc.vector.memset(ones64, 1.0)
            # persistent normalized-attention tiles; rows outside [0,97) stay
            # zero forever (never written by the per-ib mul) so the zero rows
            # of the padded out-projection weights see clean zeros.
            ao_a = consts.tile([P, IB], BF16)
            nc.vector.memset(ao_a, 0.0)
            nc.vector.memset(ao_a[96:98, :], 1.0)
            ao_b = consts.tile([P, IB], BF16)
            nc.vector.memset(ao_b, 0.0)
            # vT layout: [j-part, j-tile, head, 64]; cols 0:32 = v^T, col 32 =
            # ones (fused softmax denominator), cols 33:64 zero so the M=64 PV
            # matmul defines every PSUM row it touches.
            vT_t = qkv.tile([P, NJT, HEADS, 64], BF16)
            nc.vector.memset(vT_t[:, :, :, 33:64], 0.0)
            nc.vector.memset(vT_t[:, :, :, 32:33], 1.0)

            env = dict(
                xkv=xkv, xq=xq, out=out,
                wq_t=wq_t, wk_t=wk_t, wv_t=wv_t, woa_t=woa_t, wob_t=wob_t,
                ones64=ones64, vT_t=vT_t,
                ao_a=ao_a, ao_b=ao_b,
                xpool=xpool, qkv=qkv, epool=epool, misc=misc, opool=opool,
                psim=psim, ppv=ppv,
            )
            if reps == 1:
                _emit_body(nc, tc, mybir, env)
            else:
                with tc.For_i(0, reps, 1):
                    _emit_body(nc, tc, mybir, env)

    nc.compile()
    return nc


def _emit_body(nc, tc, mybir, env):
    F32 = mybir.dt.float32
    F32R = mybir.dt.float32r
    BF16 = mybir.dt.bfloat16
    I16 = mybir.dt.int16
    EXP = mybir.ActivationFunctionType.Exp
    MULT = mybir.AluOpType.mult
    ADD = mybir.AluOpType.add

    xkv, xq, out = env["xkv"], env["xq"], env["out"]
    wq_t, wk_t, wv_t = env["wq_t"], env["wk_t"], env["wv_t"]
    woa_t, wob_t = env["woa_t"], env["wob_t"]
    ones64, vT_t = env["ones64"], env["vT_t"]
    ao_a, ao_b = env["ao_a"], env["ao_b"]
    xpool, qkv, epool, misc, opool = (
        env["xpool"], env["qkv"], env["epool"], env["misc"], env["opool"],
    )
    psim, ppv = env["psim"], env["ppv"]

    UNIT0 = 2

    # input DMAs ride the otherwise-idle Pool queue: the next rep's loads
    # issue as soon as the projections of this rep have consumed the tiles,
    # hiding the transfer entirely behind the attention stream.
    xq_t = xpool.tile([P, 2, NI], BF16)
    nc.gpsimd.dma_start(out=xq_t, in_=xq[:].rearrange("(t p) n -> p t n", p=P))
    xkv_t = xpool.tile([P, 2, NJ], BF16)
    nc.gpsimd.dma_start(out=xkv_t, in_=xkv[:].rearrange("(t p) n -> p t n", p=P))

    q_t = qkv.tile([P, NI], BF16)
    k_t = qkv.tile([P, NJ], BF16)

    # --- projections (q / k on ACT-copy path, vT on DVE-copy path) ---
    # q/k projections, t-major: each stationary weight tile is loaded once
    # per pass and its matmuls run at streaming rate.  Chunks are packed into
    # the standard [128, 2*512] sim-tag PSUM slots (2 chunks per slot).
    pq_t = [psim.tile([P, UNIT0, IB], F32, tag="sim", name="pq") for _ in range(2)]
    for t in range(2):
        for ci, c0 in enumerate(range(0, NI, IB)):
            nc.tensor.matmul(
                pq_t[ci // 2][:, ci % 2, :],
                wq_t[:, t, :], xq_t[:, t, c0 : c0 + IB],
                start=(t == 0), stop=(t == 1),
            )
    for ci in range(2):
        nc.scalar.copy(
            q_t[:, ci * 2 * IB : (ci + 1) * 2 * IB],
            pq_t[ci][:].rearrange("p l c -> p (l c)"),
        )
    for kg in range(2):
        pk_t = [psim.tile([P, UNIT0, IB], F32, tag="sim", name="pk") for _ in range(2)]
        for t in range(2):
            for ci in range(4):
                c0 = kg * 2048 + ci * IB
                nc.tensor.matmul(
                    pk_t[ci // 2][:, ci % 2, :],
                    wk_t[:, t, :], xkv_t[:, t, c0 : c0 + IB],
                    start=(t == 0), stop=(t == 1),
                )
        for ci in range(2):
            c0 = kg * 2048 + ci * 2 * IB
            nc.scalar.copy(
                k_t[:, c0 : c0 + 2 * IB],
                pk_t[ci][:].rearrange("p l c -> p (l c)"),
            )
    for gp in range(NJT // 4):
        pvt = psim.tile([P, 4, C_HID], F32, tag="sim")
        for j4 in range(4):
            jt = gp * 4 + j4
            for t in range(2):
                nc.tensor.matmul(
                    pvt[:, j4, :], xkv_t[:, t, jt * P : (jt + 1) * P], wv_t[:, t, :],
                    start=(t == 0), stop=(t == 1),
                )
        nc.vector.tensor_copy(
            vT_t[:, gp * 4 : (gp + 1) * 4, :, 0:C_HEAD],
            pvt[:].rearrange("p j (h c) -> p j h c", h=HEADS),
        )

    # --- attention unit pipeline ---
    # chunk = (ib, jt, h) -> 512 sim columns.  unit = 2 chunks in one
    # [128, 2*512] fp32 PSUM tile (2 banks, double-buffered = 4 banks).
    # PV accumulators: 4 banks (pair x j-half), so PSUM is exactly full.
    UNIT = 2

    state = dict(pending=None, deferred=[], pv=None, pv_ib=-1, epi=None)

    def emit_qk(u):
        chunks = u["chunks"]
        sim_u = psim.tile([P, UNIT, IB], F32, tag="sim")
        for idx, (ib, jt, h) in enumerate(chunks):
            nc.tensor.matmul(
                sim_u[:, idx, :],
                k_t[h * C_HEAD : (h + 1) * C_HEAD, jt * P : (jt + 1) * P],
                q_t[h * C_HEAD : (h + 1) * C_HEAD, ib * IB : (ib + 1) * IB],
                start=True, stop=True,
                tile_position=(h * C_HEAD, 0),
            )
        u["sim"] = sim_u

    def emit_exp(u):
        sim_u = u.pop("sim")
        L = len(u["chunks"])
        e_u = epool.tile([P, UNIT, IB], BF16, tag="e")
        ncols = L * IB
        xs = int(round(ncols * ACT_FRAC / 32)) * 32
        sim_f = sim_u[:].rearrange("p l c -> p (l c)")
        e_f = e_u[:].rearrange("p l c -> p (l c)")
        if xs > 0:
            nc.scalar.activation(e_f[:, 0:xs], sim_f[:, 0:xs], EXP)
        if xs < ncols:
            nc.vector.tensor_scalar(
                e_f[:, xs:ncols].bitcast(I16), sim_f[:, xs:ncols],
                EXP_A, EXP_B, MULT, ADD,
            )
        u["e"] = e_u

    def flush_pv(macro):
        # PV contraction split into two K=64 j-halves on disjoint row groups
        # ({0,1} vs {2,3}) writing disjoint PSUM banks -> the two matmuls run
        # concurrently on the PE sub-arrays (same-cell collisions impossible).
        if macro is None:
            return
        ib0 = macro[0]["chunks"][0][0]
        if state["pv_ib"] != ib0:
            state["pv"] = [
                [
                    ppv.tile([P, IB], F32, tag=f"pv{pair}{half}", name="pv_t")
                    for half in range(2)
                ]
                for pair in range(2)
            ]
            state["pv_ib"] = ib0
        pv = state["pv"]
        for u in macro:
            for idx, (ib, jt, h) in enumerate(u["chunks"]):
                hh = h % 2
                for half in range(2):
                    nc.tensor.matmul(
                        pv[h // 2][half][64 * hh : 64 * hh + 64, :],
                        vT_t[64 * half : 64 * half + 64, jt, h, :],
                        u["e"][64 * half : 64 * half + 64, idx, :],
                        start=(jt == 0), stop=(jt == NJT - 1),
                        tile_position=(64 * half, 64 * hh),
                    )

    def epilogue_steps(ib, pv_pair):
        # generator of epilogue steps for a finished i-block
        isl = slice(ib * IB, (ib + 1) * IB)
        recips, aos, merged = [], [], []
        for pair in range(2):
            tmp = misc.tile([P, IB], BF16, tag=f"mrg{pair}", name=f"mrg{pair}")
            nc.vector.tensor_scalar(tmp[0:97, :], pv_pair[pair][0][0:97, :], 1.0, None, MULT)
            s_sb = misc.tile([P, IB], F32, tag=f"s{pair}", name=f"s_sb{pair}")
            nc.vector.tensor_add(s_sb[0:97, :], tmp[0:97, :], pv_pair[pair][1][0:97, :])
            merged.append(s_sb)
        yield
        for pair in range(2):
            recip = misc.tile([P, IB], BF16, tag=f"recip{pair}", name=f"recip{pair}")
            with nc.allow_low_precision(reason="bf16 recip feeds PE broadcast"):
                nc.vector.reciprocal(recip[0:97, :], merged[pair][0:97, :])
            recips.append(recip)
        yield
        bcs_sb = []
        for pair in range(2):
            bc_ps = psim.tile([P, IB], F32, tag="sim", name=f"bc_ps{pair}")
            for hh in range(2):
                nc.tensor.matmul(
                    bc_ps[64 * hh : 64 * hh + C_HEAD, :],
                    ones64[32 + 64 * hh : 33 + 64 * hh, :],
                    recips[pair][32 + 64 * hh : 33 + 64 * hh, :],
                    start=True, stop=True,
                    tile_position=(32 + 64 * hh, 64 * hh),
                )
            bc_sb = misc.tile([P, IB], BF16, tag=f"bc{pair}", name=f"bc_sb{pair}")
            nc.vector.tensor_scalar(bc_sb, bc_ps, 1.0, None, MULT)
            bcs_sb.append(bc_sb)
        yield
        for pair, ao in enumerate((ao_a, ao_b)):
            # rows 33:64 of merged are zeros (vT zero padding), so the garbage
            # in the unwritten bc rows is multiplied by 0; rows 97:128 of ao
            # stay at their initial 0.
            nc.vector.tensor_mul(ao[0:97, :], merged[pair][0:97, :], bcs_sb[pair][0:97, :])
            aos.append(ao)
        yield
        for ot in range(2):
            pr = ppv.tile([P, IB], F32, tag=f"pv{ot}0", name=f"pr{ot}")
            osl = slice(ot * P, (ot + 1) * P)
            nc.tensor.matmul(pr, woa_t[:, osl], aos[0], start=True, stop=False)
            nc.tensor.matmul(pr, wob_t[:, osl], aos[1], start=False, stop=True)
            o_t = opool.tile([P, IB], F32, tag="o", name="o_t")
            nc.vector.tensor_scalar(o_t, pr, 1.0, None, MULT)
            nc.sync.dma_start(
                out=out[:].rearrange("(t p) n -> p t n", p=P)[:, ot, isl],
                in_=o_t,
            )
            yield

    for ib in range(NIB):
        chunks = [(ib, jt, h) for jt in range(NJT) for h in range(HEADS)]
        units = [
            {"chunks": chunks[i : i + UNIT]} for i in range(0, len(chunks), UNIT)
        ]
        # macro-units: emit QK for two units back-to-back (4 heads on 4
        # distinct PE row groups -> 4-way concurrent), then their exps, then
        # the previous macro's PV flushes.
        for mi in range(0, len(units), 2):
            macro = units[mi : mi + 2]
            for u in macro:
                emit_qk(u)
            for u in macro:
                emit_exp(u)
            if state["epi"] is not None:
                # previous i-block's epilogue: defer PV flushes while its
                # steps interleave with our units (pv slot WAR ordering)
                state["deferred"].append(state["pending"])
                state["pending"] = None
                try:
                    next(state["epi"])
                except StopIteration:
                    state["epi"] = None
                    for d in state["deferred"]:
                        flush_pv(d)
                    state["deferred"] = []
            else:
                flush_pv(state["pending"])
            state["pending"] = macro
            if mi == 0 and ib > 0:
                # all PV of ib-1 flushed (pending was its last macro just
                # flushed above); start its epilogue now
                state["epi"] = epilogue_steps(ib - 1, state["pv"])
    # tail: flush remaining work
    flush_pv(state["pending"])
    state["pending"] = None
    if state["epi"] is not None:
        for _ in state["epi"]:
            pass
        for d in state["deferred"]:
            flush_pv(d)
        state["deferred"] = []
    for _ in epilogue_steps(NIB - 1, state["pv"]):
        pass


def _get_nc(reps=1):
    key = ("nc", reps)
    if key not in _STATE:
        _STATE[key] = _build_program(reps)
    return _STATE[key]


def _to_bf16(a):
    import ml_dtypes

    return np.ascontiguousarray(a).astype(ml_dtypes.bfloat16)


def _make_in_maps(x, w_qkv, w_out, b_out):
    x = np.ascontiguousarray(x, dtype=np.float32)
    w_qkv = np.asarray(w_qkv, dtype=np.float32)
    w_out = np.asarray(w_out, dtype=np.float32)
    b_out = np.asarray(b_out, dtype=np.float32)
    scale = np.float32(C_HEAD**-0.5)
    wo_t = w_out.T  # [c_hid, c_in]
    woa = np.zeros((C_HID, C_IN), np.float32)
    wob = np.zeros((C_HID, C_IN), np.float32)
    woa[0:32] = wo_t[0:32]  # head 0
    woa[64:96] = wo_t[32:64]  # head 1
    woa[97] = b_out  # bias rides on ao_a's persistent ones-row
    wob[0:32] = wo_t[64:96]  # head 2
    wob[64:96] = wo_t[96:128]  # head 3
    shared = {
        "wq_t": _to_bf16((w_qkv[0:C_HID] * scale).T),
        "wk_t": _to_bf16(w_qkv[C_HID : 2 * C_HID].T),
        "wv_t": _to_bf16(w_qkv[2 * C_HID : 3 * C_HID].T),
        "woa_t": _to_bf16(woa),
        "wob_t": _to_bf16(wob),
    }
    in_maps = []
    for c in range(8):
        b, half = divmod(c, 2)
        xkv = _to_bf16(x[b].reshape(C_IN, NJ))
        xq = np.ascontiguousarray(xkv[:, half * NI : (half + 1) * NI])
        in_maps.append({"xkv": xkv, "xq": xq, **shared})
    return in_maps


def _assemble(results):
    out = np.empty((B, C_IN, NJ), np.float32)
    for c in range(8):
        b, half = divmod(c, 2)
        out[b][:, half * NI : (half + 1) * NI] = results[c]["out"]
    return out.reshape(B, C_IN, 64, 64)


def _run(in_maps, reps=1, **kwargs):
    from concourse.bass_utils import run_bass_kernel_spmd

    return run_bass_kernel_spmd(
        _get_nc(reps), in_maps, core_ids=list(range(8)), **kwargs
    )


def kernel(x, w_qkv, w_out, b_out):
    res = _run(_make_in_maps(x, w_qkv, w_out, b_out))
    return _assemble(res.results)
